# revision 16
# baseline (speedup 1.0000x reference)
"""Causal self-attention Trainium2 kernel.

Problem: B=8, T=2048, C=512, H=8 heads (D=64), fp32.
  q = x@Wq.T ; k = x@Wk.T ; v = x@Wv.T  (per head)
  att = softmax(mask(q k^T / sqrt(D)))  ; y = att v ; out = y@Wp.T

Sharding: data-parallel over batch B across 8 NeuronCores (one batch
element per core, weights replicated). No collectives needed.

Per-core algorithm (everything stays on-chip; fp32r matmuls):
  - Host passes x[b].T ([C,T]) and the four W.T ([C_in,C_out]) so all
    matmuls contract over the partition dim without on-chip transposes.
  - qT/kT ([C,T]) and v ([T,C]) computed by projection matmuls.
  - Attention in "scores-transposed" layout: sT[k,q] = kT.T-block @ qT,
    exp via ScalarE (scale=1/sqrt(D) folded in, no max-subtraction --
    scores are O(1) here), causal handled by trimming whole block
    columns + one triangular mask multiply per diagonal-block pair.
  - The two heads of a pair run their QK matmuls on disjoint PE row
    groups (K=64 at base partitions 0/64) so consecutive matmuls
    overlap in the array.
  - P@V computed directly from the transposed-exp layout with a
    ones-augmented V, which also yields the softmax denominators.
  - Denominators, per head-pair (overlapped with the next pair):
    -> DRAM -> reciprocal -> DRAM -> partition-broadcast DMA -> row
    scale of yT; output projection from yT at the end.
"""

import numpy as np

import concourse.bass as bass
import concourse.bacc as bacc
import concourse.tile as tile
from concourse import mybir
from concourse.bass_utils import run_bass_kernel_spmd

B, T, C, H = 8, 2048, 512, 8
D = C // H          # 64
NT = T // 512       # 4 q-tiles of 512
NB = T // 128       # 16 k-blocks of 128
f32 = mybir.dt.float32
f32r = mybir.dt.float32r
EXP = mybir.ActivationFunctionType.Exp
N_CORES = 8


def build_nc():
    nc = bacc.Bacc(None)
    xT = nc.dram_tensor("xT", [C, T], f32r, kind="ExternalInput")
    wq = nc.dram_tensor("wqT", [C, C], f32r, kind="ExternalInput")
    wk = nc.dram_tensor("wkT", [C, C], f32r, kind="ExternalInput")
    wv = nc.dram_tensor("wvT", [C, C], f32r, kind="ExternalInput")
    wp = nc.dram_tensor("wpT", [C, C], f32r, kind="ExternalInput")
    out = nc.dram_tensor("out", [T, C], f32, kind="ExternalOutput")
    r_dram = nc.dram_tensor("r_dram", [H * NT, 512], f32)

    with tile.TileContext(nc) as tc:
        with tc.tile_pool(name="const", bufs=1) as constp, \
             tc.tile_pool(name="xw", bufs=1) as xw, \
             tc.tile_pool(name="vp", bufs=1) as vpool, \
             tc.tile_pool(name="kq", bufs=2) as kq, \
             tc.tile_pool(name="yp", bufs=1) as yp, \
             tc.tile_pool(name="expp", bufs=2) as expp, \
             tc.tile_pool(name="stg", bufs=3) as stg, \
             tc.tile_pool(name="bcp", bufs=4) as bcp, \
             tc.tile_pool(name="osb", bufs=3) as osb, \
             tc.tile_pool(name="lr", bufs=2) as lr, \
             tc.tile_pool(name="qkps", bufs=1, space="PSUM") as qkps, \
             tc.tile_pool(name="yps", bufs=1, space="PSUM") as yps, \
             tc.tile_pool(name="pps", bufs=2, space="PSUM") as pps:

            # ---- constants: [128, 256] = two copies of lower-tri keep mask
            tri = constp.tile([128, 256], f32, tag="tri")
            nc.gpsimd.memset(tri[:, :], 1.0)
            for half in range(2):
                sl = tri[:, half * 128:(half + 1) * 128]
                nc.gpsimd.affine_select(
                    out=sl, in_=sl, pattern=[[1, 128]], base=0,
                    channel_multiplier=-1,
                    compare_op=mybir.AluOpType.is_ge, fill=0.0)

            # ---- loads: small k/q weights first, then xT (so the first
            # projection matmul starts as soon as xT[0] lands), then v/p
            def load_w(dram, name):
                ws = []
                for ci in range(4):
                    t = xw.tile([128, C], f32r, tag=f"{name}{ci}")
                    nc.sync.dma_start(out=t[:, :], in_=dram[128 * ci:128 * (ci + 1), :])
                    ws.append(t)
                return ws

            xt = []
            for ci in range(4):
                t = xw.tile([128, T], f32r, tag=f"xT{ci}", name=f"xt{ci}")
                xt.append(t)

            def load_x(ci):
                nc.sync.dma_start(out=xt[ci][:, :],
                                  in_=xT[128 * ci:128 * (ci + 1), :])

            for ci in range(4):
                load_x(ci)
            wkt = load_w(wk, "wk")
            wqt = load_w(wq, "wq")
            wvt = load_w(wv, "wv")
            wpt = load_w(wp, "wp")

            def kq_proj_n(dst_t, wt, p, n):
                ps = pps.tile([128, 512], f32, tag="proj", name="pproj")
                for ci in range(4):
                    nc.tensor.matmul(
                        ps[:, :],
                        wt[ci][:, 128 * p:128 * (p + 1)],
                        xt[ci][:, 512 * n:512 * (n + 1)],
                        start=(ci == 0), stop=(ci == 3))
                nc.vector.tensor_copy(dst_t[:, 512 * n:512 * (n + 1)], ps[:, :])

            def kq_proj(dst_t, wt, p):
                for n in range(NT):
                    kq_proj_n(dst_t, wt, p, n)

            # pair-0 k/q projection tiles (filled per-qn inside the loop)
            kts = {0: kq.tile([128, T], f32r, tag="k", name="kt")}
            qts = {0: kq.tile([128, T], f32r, tag="q", name="qt")}

            # ---- V projection: v_sb[tt] = [128, 8*65], head h at cols
            # [65h, 65h+64), ones column at 65h+64. Emitted in groups of 4
            # interleaved with pair-0 attention.
            vsb = [None] * NB

            def v_proj_group(qn):
                for tt in range(4 * qn, 4 * qn + 4):
                    ps = pps.tile([128, 512], f32, tag="proj", name="pproj")
                    for ci in range(4):
                        nc.tensor.matmul(ps[:, :],
                                         xt[ci][:, 128 * tt:128 * (tt + 1)],
                                         wvt[ci][:, :],
                                         start=(ci == 0), stop=(ci == 3))
                    vt = vpool.tile([128, 8 * (D + 1)], f32r,
                                    tag=f"v{tt}", name=f"v{tt}")
                    nc.vector.memset(vt[:, :].bitcast(f32), 1.0)
                    s3 = ps[:, :].rearrange("p (h d) -> p h d", h=H)
                    dst = vt[:, :].rearrange("p (h e) -> p h e", h=H)[:, :, 0:D]
                    nc.vector.tensor_copy(dst, s3)
                    vsb[tt] = vt

            yts = [yp.tile([128, T], f32r, tag=f"yT{i}", name=f"yT{i}")
                   for i in range(4)]

            def denom_pipeline(p, qn, lsq):
                """Reciprocal + broadcast + row-scale for (head pair, q-tile)."""
                r0 = 8 * p + 2 * qn
                rsq = lr.tile([16, 64], f32, tag="rsq", name="rsq")
                nc.vector.reciprocal(out=rsq[:, :], in_=lsq[:, :])
                nc.sync.dma_start(
                    out=r_dram[r0:r0 + 2, :].rearrange("r (a b) -> (r a) b", a=8),
                    in_=rsq[:, :])
                for h in (2 * p, 2 * p + 1):
                    o = D * (h % 2)
                    r = r0 + (h % 2)
                    bt = bcp.tile([128, 512], f32, tag="bc", name="bc")
                    nc.sync.dma_start(
                        out=bt[o:o + D, :],
                        in_=r_dram[r:r + 1, :].to_broadcast([D, 512]))
                    ysl = yts[p][o:o + D, 512 * qn:512 * (qn + 1)]
                    nc.vector.tensor_mul(ysl, ysl.bitcast(f32), bt[o:o + D, :])

            # ---- per head-pair attention (both heads interleaved so their
            # K=64 QK matmuls land on disjoint PE row groups back-to-back)
            for p in range(4):
                if p > 0:
                    kts[p] = kq.tile([128, T], f32r, tag="k", name="kt")
                    qts[p] = kq.tile([128, T], f32r, tag="q", name="qt")
                    kq_proj(kts[p], wkt, p)
                    kq_proj(qts[p], wqt, p)
                kt = kts[p]
                qt_ = qts[p]
                hA, hB = 2 * p, 2 * p + 1
                for qn in range(NT):
                    if p == 0:
                        kq_proj_n(kt, wkt, 0, qn)
                        kq_proj_n(qt_, wqt, 0, qn)
                    q0 = 512 * qn
                    nblocks = 4 * qn + 4
                    ypsA = yps.tile([D + 1, 512], f32, tag="yA", name="ypsA")
                    ypsB = yps.tile([D + 1, 512], f32, tag="yB", name="ypsB")
                    for c in range(nblocks // 2):
                        qkA = qkps.tile([128, 1024], f32, tag="qkA", name="qkA")
                        qkB = qkps.tile([128, 1024], f32, tag="qkB", name="qkB")
                        exA = expp.tile([128, 1024], f32r, tag="exA", name="exA")
                        exB = expp.tile([128, 1024], f32r, tag="exB", name="exB")
                        ms = [max(0, 2 * c + u - 4 * qn) for u in (0, 1)]
                        for u in (0, 1):
                            j = 2 * c + u
                            mcs = 128 * min(ms[u], 2)  # matmul N >= 256
                            for o, qk in ((0, qkA), (D, qkB)):
                                nc.tensor.matmul(
                                    qk[:, 512 * u + mcs:512 * (u + 1)],
                                    kt[o:o + D, 128 * j:128 * (j + 1)],
                                    qt_[o:o + D, q0 + mcs:q0 + 512],
                                    start=True, stop=True)
                        for qk, ex in ((qkA, exA), (qkB, exB)):
                            if ms[1] == 0:  # both blocks fully valid
                                nc.scalar.activation(out=ex[:, :], in_=qk[:, :],
                                                     func=EXP, scale=0.125)
                            else:
                                for u in (0, 1):
                                    cs = 128 * ms[u]
                                    nc.scalar.activation(
                                        out=ex[:, 512 * u + cs:512 * (u + 1)],
                                        in_=qk[:, 512 * u + cs:512 * (u + 1)],
                                        func=EXP, scale=0.125)
                                if ms[1] == 3:
                                    # PV below reads cols 768:896; not
                                    # written by exp -> zero them
                                    nc.vector.memset(
                                        ex[:, 768:896].bitcast(f32), 0.0)
                                st = 128 * ms[0]
                                src = ex[:, st:st + 128]
                                ap3 = bass.AP(
                                    tensor=src.tensor, offset=src.offset,
                                    ap=[src.ap[0], [640, 2], [1, 128]])
                                tri3 = tri[:, :].rearrange("p (a b) -> p a b", a=2)
                                nc.vector.tensor_mul(ap3.bitcast(f32r),
                                                     ap3.bitcast(f32),
                                                     tri3)
                        if p == 0 and c == 0:
                            v_proj_group(qn)
                        for u in (0, 1):
                            j = 2 * c + u
                            mcs = 128 * min(ms[u], 2)
                            for h, yps_t, ex in ((hA, ypsA, exA), (hB, ypsB, exB)):
                                nc.tensor.matmul(
                                    yps_t[0:D + 1, mcs:512],
                                    vsb[j][:, 65 * h:65 * h + 65],
                                    ex[:, 512 * u + mcs:512 * (u + 1)],
                                    start=(j == 0), stop=(j == nblocks - 1))
                    lsq = lr.tile([16, 64], f32, tag="lsq", name="lsq")
                    for h, yps_t in ((hA, ypsA), (hB, ypsB)):
                        o = D * (h % 2)
                        stt = stg.tile([D + 1, 512], f32r, tag="st", name="stt")
                        nc.vector.tensor_copy(stt[:, :], yps_t[0:D + 1, :])
                        nc.sync.dma_start(
                            out=lsq[8 * (h % 2):8 * (h % 2) + 8, :],
                            in_=stt[D:D + 1, :].bitcast(f32))
                        nc.sync.dma_start(
                            out=yts[p][o:o + D, q0:q0 + 512],
                            in_=stt[0:D, :])
                    denom_pipeline(p, qn, lsq)

            # ---- output projection
            for tt in range(NB):
                ps = pps.tile([128, 512], f32, tag="proj", name="pproj")
                for ci in range(4):
                    nc.tensor.matmul(ps[:, :],
                                     yts[ci][:, 128 * tt:128 * (tt + 1)],
                                     wpt[ci][:, :],
                                     start=(ci == 0), stop=(ci == 3))
                ot = osb.tile([128, 512], f32, tag="o", name="ot")
                nc.scalar.copy(ot[:, :], ps[:, :])
                nc.sync.dma_start(out=out[128 * tt:128 * (tt + 1), :], in_=ot[:, :])

    nc.compile()
    return nc


_NC = None


def _get_nc():
    global _NC
    if _NC is None:
        _NC = build_nc()
    return _NC


def _round_f32r(a: np.ndarray) -> np.ndarray:
    """Round fp32 to fp32r (11-bit mantissa) with round-to-nearest."""
    a = np.ascontiguousarray(a, dtype=np.float32)
    u = a.view(np.uint32).astype(np.uint64)
    u = (u + 0x800) & 0xFFFFF000
    return u.astype(np.uint32).view(np.float32)


def kernel(**inputs: np.ndarray) -> np.ndarray:
    x = np.asarray(inputs["x"], dtype=np.float32)
    wqT = _round_f32r(np.asarray(inputs["Wq"], dtype=np.float32).T)
    wkT = _round_f32r(np.asarray(inputs["Wk"], dtype=np.float32).T)
    wvT = _round_f32r(np.asarray(inputs["Wv"], dtype=np.float32).T)
    wpT = _round_f32r(np.asarray(inputs["Wp"], dtype=np.float32).T)
    nc = _get_nc()
    in_maps = []
    for b in range(N_CORES):
        in_maps.append({
            "xT": _round_f32r(x[b].T),
            "wqT": wqT, "wkT": wkT, "wvT": wvT, "wpT": wpT,
        })
    res = run_bass_kernel_spmd(nc, in_maps, core_ids=list(range(N_CORES)))
    return np.stack([res.results[b]["out"] for b in range(N_CORES)], axis=0)


if __name__ == "__main__":
    nc = _get_nc()
    from concourse.timeline_sim import TimelineSim
    print("TimelineSim predicted ns:", TimelineSim(nc).simulate())


# revision 17
# speedup vs baseline: 1.0264x; 1.0264x over previous
"""Causal self-attention Trainium2 kernel.

Problem: B=8, T=2048, C=512, H=8 heads (D=64), fp32.
  q = x@Wq.T ; k = x@Wk.T ; v = x@Wv.T  (per head)
  att = softmax(mask(q k^T / sqrt(D)))  ; y = att v ; out = y@Wp.T

Sharding: data-parallel over batch B across 8 NeuronCores (one batch
element per core, weights replicated). No collectives needed.

Per-core algorithm (everything stays on-chip; fp32r matmuls):
  - Host passes x[b].T ([C,T]) and the four W.T ([C_in,C_out]) so all
    matmuls contract over the partition dim without on-chip transposes.
  - qT/kT ([C,T]) and v ([T,C]) computed by projection matmuls.
  - Attention in "scores-transposed" layout: sT[k,q] = kT.T-block @ qT,
    exp via ScalarE (scale=1/sqrt(D) folded in, no max-subtraction --
    scores are O(1) here), causal handled by trimming whole block
    columns + one triangular mask multiply per diagonal-block pair.
  - The two heads of a pair run their QK matmuls on disjoint PE row
    groups (K=64 at base partitions 0/64) so consecutive matmuls
    overlap in the array.
  - P@V computed directly from the transposed-exp layout with a
    ones-augmented V, which also yields the softmax denominators.
  - Denominators, per head-pair (overlapped with the next pair):
    -> DRAM -> reciprocal -> DRAM -> partition-broadcast DMA -> row
    scale of yT; output projection from yT at the end.
"""

import numpy as np

import concourse.bass as bass
import concourse.bacc as bacc
import concourse.tile as tile
from concourse import mybir
from concourse.bass_utils import run_bass_kernel_spmd

B, T, C, H = 8, 2048, 512, 8
D = C // H          # 64
NT = T // 512       # 4 q-tiles of 512
NB = T // 128       # 16 k-blocks of 128
f32 = mybir.dt.float32
f32r = mybir.dt.float32r
EXP = mybir.ActivationFunctionType.Exp
N_CORES = 8


def build_nc():
    nc = bacc.Bacc(None)
    xT = nc.dram_tensor("xT", [C, T], f32r, kind="ExternalInput")
    wq = nc.dram_tensor("wqT", [C, C], f32r, kind="ExternalInput")
    wk = nc.dram_tensor("wkT", [C, C], f32r, kind="ExternalInput")
    wv = nc.dram_tensor("wvT", [C, C], f32r, kind="ExternalInput")
    wp = nc.dram_tensor("wpT", [C, C], f32r, kind="ExternalInput")
    out = nc.dram_tensor("out", [T, C], f32, kind="ExternalOutput")
    r_dram = nc.dram_tensor("r_dram", [H * NT, 512], f32)

    with tile.TileContext(nc) as tc:
        with tc.tile_pool(name="const", bufs=1) as constp, \
             tc.tile_pool(name="xw", bufs=1) as xw, \
             tc.tile_pool(name="vp", bufs=1) as vpool, \
             tc.tile_pool(name="kq", bufs=2) as kq, \
             tc.tile_pool(name="yp", bufs=1) as yp, \
             tc.tile_pool(name="expp", bufs=2) as expp, \
             tc.tile_pool(name="stg", bufs=3) as stg, \
             tc.tile_pool(name="bcp", bufs=4) as bcp, \
             tc.tile_pool(name="osb", bufs=3) as osb, \
             tc.tile_pool(name="lr", bufs=2) as lr, \
             tc.tile_pool(name="qkps", bufs=1, space="PSUM") as qkps, \
             tc.tile_pool(name="yps", bufs=1, space="PSUM") as yps, \
             tc.tile_pool(name="pps", bufs=2, space="PSUM") as pps:

            # ---- constants: [128, 256] = two copies of lower-tri keep mask
            tri = constp.tile([128, 256], f32, tag="tri")
            nc.gpsimd.memset(tri[:, :], 1.0)
            for half in range(2):
                sl = tri[:, half * 128:(half + 1) * 128]
                nc.gpsimd.affine_select(
                    out=sl, in_=sl, pattern=[[1, 128]], base=0,
                    channel_multiplier=-1,
                    compare_op=mybir.AluOpType.is_ge, fill=0.0)

            # ---- loads: small k/q weights first, then xT (so the first
            # projection matmul starts as soon as xT[0] lands), then v/p
            def load_w(dram, name):
                ws = []
                for ci in range(4):
                    t = xw.tile([128, C], f32r, tag=f"{name}{ci}")
                    nc.sync.dma_start(out=t[:, :], in_=dram[128 * ci:128 * (ci + 1), :])
                    ws.append(t)
                return ws

            xt = []
            for ci in range(4):
                t = xw.tile([128, T], f32r, tag=f"xT{ci}", name=f"xt{ci}")
                xt.append(t)

            def load_x_n(n):
                for ci in range(4):
                    nc.sync.dma_start(
                        out=xt[ci][:, 512 * n:512 * (n + 1)],
                        in_=xT[128 * ci:128 * (ci + 1), 512 * n:512 * (n + 1)])

            # interleave loads so the first k/q projection (needs wk/wq +
            # x columns 0:512 only) can start ~5us in, not after all of xT
            wkt = load_w(wk, "wk")
            load_x_n(0)
            wqt = load_w(wq, "wq")
            load_x_n(1)
            wvt = load_w(wv, "wv")
            load_x_n(2)
            load_x_n(3)
            wpt = load_w(wp, "wp")

            def kq_proj_n(dst_t, wt, p, n):
                ps = pps.tile([128, 512], f32, tag="proj", name="pproj")
                for ci in range(4):
                    nc.tensor.matmul(
                        ps[:, :],
                        wt[ci][:, 128 * p:128 * (p + 1)],
                        xt[ci][:, 512 * n:512 * (n + 1)],
                        start=(ci == 0), stop=(ci == 3))
                nc.vector.tensor_copy(dst_t[:, 512 * n:512 * (n + 1)], ps[:, :])

            def kq_proj(dst_t, wt, p):
                for n in range(NT):
                    kq_proj_n(dst_t, wt, p, n)

            # pair-0 k/q projection tiles (filled per-qn inside the loop)
            kts = {0: kq.tile([128, T], f32r, tag="k", name="kt")}
            qts = {0: kq.tile([128, T], f32r, tag="q", name="qt")}

            # ---- V projection: v_sb[tt] = [128, 8*65], head h at cols
            # [65h, 65h+64), ones column at 65h+64. Emitted in groups of 4
            # interleaved with pair-0 attention.
            vsb = [None] * NB

            def v_proj_group(qn):
                for tt in range(4 * qn, 4 * qn + 4):
                    ps = pps.tile([128, 512], f32, tag="proj", name="pproj")
                    for ci in range(4):
                        nc.tensor.matmul(ps[:, :],
                                         xt[ci][:, 128 * tt:128 * (tt + 1)],
                                         wvt[ci][:, :],
                                         start=(ci == 0), stop=(ci == 3))
                    vt = vpool.tile([128, 8 * (D + 1)], f32r,
                                    tag=f"v{tt}", name=f"v{tt}")
                    nc.vector.memset(vt[:, :].bitcast(f32), 1.0)
                    s3 = ps[:, :].rearrange("p (h d) -> p h d", h=H)
                    dst = vt[:, :].rearrange("p (h e) -> p h e", h=H)[:, :, 0:D]
                    nc.vector.tensor_copy(dst, s3)
                    vsb[tt] = vt

            yts = [yp.tile([128, T], f32r, tag=f"yT{i}", name=f"yT{i}")
                   for i in range(4)]

            def denom_pipeline(p, qn, lsq):
                """Reciprocal + broadcast + row-scale for (head pair, q-tile)."""
                r0 = 8 * p + 2 * qn
                rsq = lr.tile([16, 64], f32, tag="rsq", name="rsq")
                nc.vector.reciprocal(out=rsq[:, :], in_=lsq[:, :])
                nc.sync.dma_start(
                    out=r_dram[r0:r0 + 2, :].rearrange("r (a b) -> (r a) b", a=8),
                    in_=rsq[:, :])
                for h in (2 * p, 2 * p + 1):
                    o = D * (h % 2)
                    r = r0 + (h % 2)
                    bt = bcp.tile([128, 512], f32, tag="bc", name="bc")
                    nc.sync.dma_start(
                        out=bt[o:o + D, :],
                        in_=r_dram[r:r + 1, :].to_broadcast([D, 512]))
                    ysl = yts[p][o:o + D, 512 * qn:512 * (qn + 1)]
                    nc.vector.tensor_mul(ysl, ysl.bitcast(f32), bt[o:o + D, :])

            # ---- per head-pair attention (both heads interleaved so their
            # K=64 QK matmuls land on disjoint PE row groups back-to-back)
            for p in range(4):
                if p > 0:
                    kts[p] = kq.tile([128, T], f32r, tag="k", name="kt")
                    qts[p] = kq.tile([128, T], f32r, tag="q", name="qt")
                    kq_proj(kts[p], wkt, p)
                    kq_proj(qts[p], wqt, p)
                kt = kts[p]
                qt_ = qts[p]
                hA, hB = 2 * p, 2 * p + 1
                for qn in range(NT):
                    if p == 0:
                        kq_proj_n(kt, wkt, 0, qn)
                        kq_proj_n(qt_, wqt, 0, qn)
                    q0 = 512 * qn
                    nblocks = 4 * qn + 4
                    ypsA = yps.tile([D + 1, 512], f32, tag="yA", name="ypsA")
                    ypsB = yps.tile([D + 1, 512], f32, tag="yB", name="ypsB")
                    for c in range(nblocks // 2):
                        qkA = qkps.tile([128, 1024], f32, tag="qkA", name="qkA")
                        qkB = qkps.tile([128, 1024], f32, tag="qkB", name="qkB")
                        exA = expp.tile([128, 1024], f32r, tag="exA", name="exA")
                        exB = expp.tile([128, 1024], f32r, tag="exB", name="exB")
                        ms = [max(0, 2 * c + u - 4 * qn) for u in (0, 1)]
                        for u in (0, 1):
                            j = 2 * c + u
                            mcs = 128 * min(ms[u], 2)  # matmul N >= 256
                            for o, qk in ((0, qkA), (D, qkB)):
                                nc.tensor.matmul(
                                    qk[:, 512 * u + mcs:512 * (u + 1)],
                                    kt[o:o + D, 128 * j:128 * (j + 1)],
                                    qt_[o:o + D, q0 + mcs:q0 + 512],
                                    start=True, stop=True)
                        for qk, ex in ((qkA, exA), (qkB, exB)):
                            if ms[1] == 0:  # both blocks fully valid
                                nc.scalar.activation(out=ex[:, :], in_=qk[:, :],
                                                     func=EXP, scale=0.125)
                            else:
                                for u in (0, 1):
                                    cs = 128 * ms[u]
                                    nc.scalar.activation(
                                        out=ex[:, 512 * u + cs:512 * (u + 1)],
                                        in_=qk[:, 512 * u + cs:512 * (u + 1)],
                                        func=EXP, scale=0.125)
                                if ms[1] == 3:
                                    # PV below reads cols 768:896; not
                                    # written by exp -> zero them
                                    nc.vector.memset(
                                        ex[:, 768:896].bitcast(f32), 0.0)
                                st = 128 * ms[0]
                                src = ex[:, st:st + 128]
                                ap3 = bass.AP(
                                    tensor=src.tensor, offset=src.offset,
                                    ap=[src.ap[0], [640, 2], [1, 128]])
                                tri3 = tri[:, :].rearrange("p (a b) -> p a b", a=2)
                                nc.vector.tensor_mul(ap3.bitcast(f32r),
                                                     ap3.bitcast(f32),
                                                     tri3)
                        if p == 0 and c == 0:
                            v_proj_group(qn)
                        for u in (0, 1):
                            j = 2 * c + u
                            mcs = 128 * min(ms[u], 2)
                            for h, yps_t, ex in ((hA, ypsA, exA), (hB, ypsB, exB)):
                                nc.tensor.matmul(
                                    yps_t[0:D + 1, mcs:512],
                                    vsb[j][:, 65 * h:65 * h + 65],
                                    ex[:, 512 * u + mcs:512 * (u + 1)],
                                    start=(j == 0), stop=(j == nblocks - 1))
                    lsq = lr.tile([16, 64], f32, tag="lsq", name="lsq")
                    for h, yps_t in ((hA, ypsA), (hB, ypsB)):
                        o = D * (h % 2)
                        stt = stg.tile([D + 1, 512], f32r, tag="st", name="stt")
                        nc.vector.tensor_copy(stt[:, :], yps_t[0:D + 1, :])
                        nc.sync.dma_start(
                            out=lsq[8 * (h % 2):8 * (h % 2) + 8, :],
                            in_=stt[D:D + 1, :].bitcast(f32))
                        nc.sync.dma_start(
                            out=yts[p][o:o + D, q0:q0 + 512],
                            in_=stt[0:D, :])
                    denom_pipeline(p, qn, lsq)

            # ---- output projection
            for tt in range(NB):
                ps = pps.tile([128, 512], f32, tag="proj", name="pproj")
                for ci in range(4):
                    nc.tensor.matmul(ps[:, :],
                                     yts[ci][:, 128 * tt:128 * (tt + 1)],
                                     wpt[ci][:, :],
                                     start=(ci == 0), stop=(ci == 3))
                ot = osb.tile([128, 512], f32, tag="o", name="ot")
                nc.scalar.copy(ot[:, :], ps[:, :])
                nc.sync.dma_start(out=out[128 * tt:128 * (tt + 1), :], in_=ot[:, :])

    nc.compile()
    return nc


_NC = None


def _get_nc():
    global _NC
    if _NC is None:
        _NC = build_nc()
    return _NC


def _round_f32r(a: np.ndarray) -> np.ndarray:
    """Round fp32 to fp32r (11-bit mantissa) with round-to-nearest."""
    a = np.ascontiguousarray(a, dtype=np.float32)
    u = a.view(np.uint32).astype(np.uint64)
    u = (u + 0x800) & 0xFFFFF000
    return u.astype(np.uint32).view(np.float32)


def kernel(**inputs: np.ndarray) -> np.ndarray:
    x = np.asarray(inputs["x"], dtype=np.float32)
    wqT = _round_f32r(np.asarray(inputs["Wq"], dtype=np.float32).T)
    wkT = _round_f32r(np.asarray(inputs["Wk"], dtype=np.float32).T)
    wvT = _round_f32r(np.asarray(inputs["Wv"], dtype=np.float32).T)
    wpT = _round_f32r(np.asarray(inputs["Wp"], dtype=np.float32).T)
    nc = _get_nc()
    in_maps = []
    for b in range(N_CORES):
        in_maps.append({
            "xT": _round_f32r(x[b].T),
            "wqT": wqT, "wkT": wkT, "wvT": wvT, "wpT": wpT,
        })
    res = run_bass_kernel_spmd(nc, in_maps, core_ids=list(range(N_CORES)))
    return np.stack([res.results[b]["out"] for b in range(N_CORES)], axis=0)


if __name__ == "__main__":
    nc = _get_nc()
    from concourse.timeline_sim import TimelineSim
    print("TimelineSim predicted ns:", TimelineSim(nc).simulate())
